# revision 37
# baseline (speedup 1.0000x reference)
"""Causal self-attention (B=4, T=2048, C=1024, H=16) on 8 trn2 NeuronCores.

Sharding: core c -> batch b = c//2, heads h0 = (c%2)*8 .. h0+8 (tensor
parallel over heads: c_attn columns / c_proj rows split). Each core computes a
partial projection output [T, C] in bf16; the host sums the two partials per
batch and adds b_proj.

v2: single interleaved emission schedule so ScalarE exp overlaps all phases:
  - qkv projections chunked to [128,512] PSUM accumulators (1 bank each)
  - attention per (head, T/2-half): S^T superchunks [128,<=1024] -> exp ->
    PV accumulate into per-half yt [MV,1024] PSUM tile
  - v-stage / next-m qk-stage / proj chunks emitted as PE filler between
    attention chunks (keeps PE busy while exp runs, starts exp at ~25us)
  - output partials in bf16 (halves output DMA; rel-err ~4e-3 total)
"""

import numpy as np

P = 128


def _bf16_np():
    import ml_dtypes
    return ml_dtypes.bfloat16


def build_program(T=2048, C=1024, HC=8, D=64, num_devices=8, trn="TRN2"):
    import concourse.mybir as mybir
    import concourse.tile as tile
    from concourse import bacc
    from concourse.masks import make_upper_triangular

    W = 512          # matmul moving-dim chunk
    KC = C // P      # contraction tiles over C (8)
    CO = HC * D      # this core's qkv channel block (512)
    NP = CO // P     # head pairs per core (4)
    TT = T // P      # T tiles (16)
    T2 = T // 2      # half length (1024)
    NC512 = T // W   # 512-chunks across T (4)
    MV = 80          # PV stationary columns: D v-cols + 1 ones + pad
    dt32 = mybir.dt.float32
    f32r = mybir.dt.float32r
    bf16 = mybir.dt.bfloat16
    ActF = mybir.ActivationFunctionType
    Alu = mybir.AluOpType
    scale = 1.0 / float(np.sqrt(D))

    nc = bacc.Bacc(trn, target_bir_lowering=False, debug=False,
                   enable_asserts=False, num_devices=num_devices)

    xt_d = nc.dram_tensor("xt", [C, T], bf16, kind="ExternalInput")
    wq_d = nc.dram_tensor("wq", [C, CO], bf16, kind="ExternalInput")
    wk_d = nc.dram_tensor("wk", [C, CO], bf16, kind="ExternalInput")
    wv_d = nc.dram_tensor("wv", [C, CO], bf16, kind="ExternalInput")
    bq_d = nc.dram_tensor("bq", [P, NP], dt32, kind="ExternalInput")
    bk_d = nc.dram_tensor("bk", [P, NP], dt32, kind="ExternalInput")
    bvb_d = nc.dram_tensor("bvb", [P, CO], dt32, kind="ExternalInput")
    ones_d = nc.dram_tensor("ones", [P, TT * HC], bf16, kind="ExternalInput")
    wp_d = nc.dram_tensor("wp", [CO, C], f32r, kind="ExternalInput")
    out_d = nc.dram_tensor("out", [T, C], bf16, kind="ExternalOutput")
    lsc_d = nc.dram_tensor("lsc", [HC * T], dt32)
    lsc2_d = nc.dram_tensor("lsc2", [HC * T], dt32)

    with tile.TileContext(nc) as tc:
        with tc.tile_pool(name="const", bufs=1) as cpool, \
             tc.tile_pool(name="pers", bufs=1) as pers, \
             tc.tile_pool(name="wts", bufs=3 * KC) as wpool, \
             tc.tile_pool(name="wpj", bufs=1) as wppool, \
             tc.tile_pool(name="xtp", bufs=KC) as xpool, \
             tc.tile_pool(name="ptp", bufs=4) as ptpool, \
             tc.tile_pool(name="ysp", bufs=2) as yspool, \
             tc.tile_pool(name="nrm", bufs=2) as nrmpool, \
             tc.tile_pool(name="ost", bufs=3) as opool, \
             tc.tile_pool(name="psX", bufs=2, space="PSUM") as psX, \
             tc.tile_pool(name="psS", bufs=2, space="PSUM") as psS, \
             tc.tile_pool(name="psY", bufs=2, space="PSUM") as psY:

            # ---- constants, persistent tensors, prefetch DMAs ----
            tri = cpool.tile([P, P], bf16)
            make_upper_triangular(nc, tri[:], val=1.0, diag=True)
            bq_sb = cpool.tile([P, NP], dt32)
            nc.scalar.dma_start(bq_sb[:], bq_d.ap())
            bk_sb = cpool.tile([P, NP], dt32)
            nc.scalar.dma_start(bk_sb[:], bk_d.ap())
            bvb_sb = cpool.tile([P, CO], dt32)
            nc.scalar.dma_start(bvb_sb[:], bvb_d.ap())
            bvb_v = bvb_sb[:].rearrange("p (h d) -> p h d", d=D)

            qT = pers.tile([P, NP, T], bf16, tag="qT")
            kT = pers.tile([P, NP, T], bf16, tag="kT")  # packed like qT
            vaug = pers.tile([P, TT, HC, MV], bf16, tag="vaug")
            yT = pers.tile([P, NP, T], f32r, tag="yT")
            nc.vector.memset(vaug[:], 0.0)
            nc.scalar.dma_start(
                vaug[:, :, :, D],
                ones_d.ap().rearrange("p (a b) -> p a b", b=HC))

            xt_view = xt_d.ap().rearrange("(kc p) t -> kc p t", p=P)
            dmae = [nc.sync, nc.gpsimd, nc.scalar]
            di = 0

            def dma(dst, src, n=3):
                nonlocal di
                dmae[di % n].dma_start(dst, src)
                di += 1

            # DMA order: per kc [wq, x cols 0:T2, wk] so the first qk
            # chunks (c0/c1) start after ~4MB instead of ~6MB; then wv
            # (v_tt fillers) and the second x half (c2/c3 + v tt>=8)
            wq_t, wk_t, xbig = [], [], []
            wq_view = wq_d.ap().rearrange("(kc p) n -> kc p n", p=P)
            wk_view = wk_d.ap().rearrange("(kc p) n -> kc p n", p=P)
            for kc in range(KC):
                wt = wpool.tile([P, CO], bf16, tag="w")
                dma(wt[:], wq_view[kc])
                wq_t.append(wt)
                xb = xpool.tile([P, T], bf16, tag="xt")
                dma(xb[:, 0:T2], xt_view[kc][:, 0:T2])
                xbig.append(xb)
                wt2 = wpool.tile([P, CO], bf16, tag="w")
                dma(wt2[:], wk_view[kc])
                wk_t.append(wt2)
            xts = [[xbig[kc][:, c * W:(c + 1) * W] for c in range(NC512)]
                   for kc in range(KC)]

            def load_w(w_d):
                view = w_d.ap().rearrange("(kc p) n -> kc p n", p=P)
                tiles = []
                for kc in range(KC):
                    wt = wpool.tile([P, CO], bf16, tag="w")
                    dma(wt[:], view[kc])
                    tiles.append(wt)
                return tiles

            wv_t = load_w(wv_d)
            for kc in range(KC):
                dma(xbig[kc][:, T2:T], xt_view[kc][:, T2:T])
            wpsb = wppool.tile([P, NP, C], f32r)

            # ---- stage emitters (each a small closure; order = schedule) ----
            def q_chunk(m, c):
                ps = psX.tile([P, W], dt32, tag="x")
                for kc in range(KC):
                    nc.tensor.matmul(
                        ps[:], wq_t[kc][:, m * P:(m + 1) * P], xts[kc][c][:],
                        start=(kc == 0), stop=(kc == KC - 1),
                        skip_group_check=True)
                nc.scalar.activation(
                    qT[:, m, c * W:(c + 1) * W], ps[:],
                    ActF.Identity, bias=bq_sb[:, m:m + 1], scale=1.0)

            def k_chunk(m, c):
                ps = psX.tile([P, W], dt32, tag="x")
                for kc in range(KC):
                    nc.tensor.matmul(
                        ps[:], wk_t[kc][:, m * P:(m + 1) * P], xts[kc][c][:],
                        start=(kc == 0), stop=(kc == KC - 1),
                        skip_group_check=True)
                nc.scalar.activation(
                    kT[:, m, c * W:(c + 1) * W], ps[:],
                    ActF.Identity, bias=bk_sb[:, m:m + 1], scale=1.0)

            def v_tt(tt):
                ps = psX.tile([P, CO], dt32, tag="x")
                c, off = (tt * P) // W, (tt * P) % W
                for kc in range(KC):
                    nc.tensor.matmul(
                        ps[:], xts[kc][c][:, off:off + P], wv_t[kc][:],
                        start=(kc == 0), stop=(kc == KC - 1),
                        skip_group_check=True)
                nc.vector.scalar_tensor_tensor(
                    out=vaug[:, tt, :, 0:D],
                    in0=ps[:].rearrange("p (h d) -> p h d", d=D),
                    scalar=1.0, in1=bvb_v,
                    op0=Alu.mult, op1=Alu.add)

            def s_chunk(m, qq, j):
                # S^T for BOTH heads of pair m, q-chunk qq (cols
                # [qq*W, qq*W+W)), kpos tile j. K=64 row-tiled matmuls:
                # even head in array rows 0:63 (tile T0), odd in 64:127
                # (tile T8) -- they execute concurrently. One exp covers
                # both heads' scores.
                jb = j * P
                lo = qq * W
                q0 = max(jb, lo)
                w = lo + W - q0
                pt = ptpool.tile([P, 2, w], bf16, tag="pt")
                sps = psS.tile([P, 2, W], dt32, tag="s")
                o = q0 - lo
                nc.tensor.matmul(
                    sps[:, 0, o:o + w],
                    kT[0:D, m, jb:jb + P],
                    qT[0:D, m, q0:q0 + w],
                    start=True, stop=True, skip_group_check=True)
                nc.tensor.matmul(
                    sps[:, 1, o:o + w],
                    kT[D:P, m, jb:jb + P],
                    qT[D:P, m, q0:q0 + w],
                    start=True, stop=True, skip_group_check=True)
                nc.scalar.activation(
                    pt[:], sps[:, :, o:o + w], ActF.Exp, scale=scale)
                if q0 == jb:
                    nc.vector.tensor_mul(pt[:, 0, 0:P], pt[:, 0, 0:P], tri[:])
                    nc.vector.tensor_mul(pt[:, 1, 0:P], pt[:, 1, 0:P], tri[:])
                return pt

            def pv_chunk(m, qq, j, pt, yts):
                # yts = [yt_even, yt_odd], each [MV, W] (one PSUM bank)
                jb = j * P
                lo = qq * W
                q0 = max(jb, lo)
                w = lo + W - q0
                jmax = (lo + W) // P - 1
                for hh in (0, 1):
                    nc.tensor.matmul(
                        yts[hh][:, q0 - lo:q0 - lo + w],
                        vaug[:, j, 2 * m + hh, :],
                        pt[:, hh, 0:w],
                        start=(j == 0), stop=(j == jmax),
                        skip_group_check=True)

            def finish(h, qq, yt):
                # normalize quarter qq (columns [qq*W, qq*W+W)) of head h:
                # evacuate PSUM, reciprocal of the ones-row, gpsimd
                # partition-broadcast (no DRAM round-trip), multiply
                m, r0 = h // 2, (h % 2) * D
                lo = qq * W
                dlo = h * T + lo  # per-head DRAM scratch region
                ys = yspool.tile([D + 1, W], dt32, tag="ys")
                nc.vector.tensor_copy(ys[:], yt[0:D + 1, :])
                nc.sync.dma_start(
                    lsc_d.ap()[dlo:dlo + W].rearrange("(o t) -> o t", o=1),
                    ys[D:D + 1, :])
                l128 = nrmpool.tile([P, W // P], dt32, tag="l128")
                nc.gpsimd.dma_start(
                    l128[:],
                    lsc_d.ap()[dlo:dlo + W].rearrange("(p c) -> p c", p=P))
                nc.vector.reciprocal(l128[:], l128[:])
                nc.gpsimd.dma_start(
                    lsc2_d.ap()[dlo:dlo + W].rearrange("(p c) -> p c", p=P),
                    l128[:])
                bc = nrmpool.tile([D, W], dt32, tag="bc")
                nc.sync.dma_start(
                    bc[:],
                    lsc2_d.ap()[dlo:dlo + W].rearrange(
                        "(o t) -> o t", o=1).broadcast_to([D, W]))
                nc.vector.tensor_mul(
                    yT[r0:r0 + D, m, lo:lo + W], ys[0:D, :], bc[:])

            def proj_chunk(tt, nn):
                po = psX.tile([P, W], dt32, tag="x")
                for kt in range(NP):
                    nc.tensor.matmul(
                        po[:], yT[:, kt, tt * P:(tt + 1) * P],
                        wpsb[:, kt, nn * W:(nn + 1) * W],
                        start=(kt == 0), stop=(kt == NP - 1),
                        skip_group_check=True)
                ot = opool.tile([P, W], bf16, tag="ot")
                nc.vector.tensor_copy(ot[:], po[:])
                dma(out_d.ap()[tt * P:(tt + 1) * P, nn * W:(nn + 1) * W],
                    ot[:], n=2)

            # ---- the schedule ----
            # One global S->exp->PV pipeline over pair-chunks (m, qq, j),
            # qq = 512-col q-quarter, j = kpos tile (j <= 4qq+3). PV trails
            # S by DEPTH chunks; each (head, quarter) accumulator finishes
            # as soon as its last PV lands (staggered PSUM release; proj
            # overlaps the final pair). Fillers (v/qk/proj) paced per pair
            # window so PE has work wherever exp is dense.
            def pair_fillers(m):
                items = []
                if m == 0:
                    items = [lambda tt=tt: v_tt(tt) for tt in range(TT)]
                    for c in range(NC512):
                        items.append(lambda c=c: q_chunk(1, c))
                        items.append(lambda c=c: k_chunk(1, c))
                elif m in (1, 2):
                    for c in range(NC512):
                        items.append(lambda c=c: q_chunk(m + 1, c))
                        items.append(lambda c=c: k_chunk(m + 1, c))
                    if m == 2:
                        items.append(lambda: nc.scalar.dma_start(
                            wpsb[:],
                            wp_d.ap().rearrange("(kt p) n -> p kt n", p=P)))
                else:
                    items = [lambda t=t: proj_chunk(t // 2, t % 2)
                             for t in range(24)]
                return items

            def wanted(m, i, n):
                if m == 0:
                    # v(tt) by chunk tt; qk(m1) over the rest
                    return min(i + 1, 16) + max(0, ((i - 15) * 8) // 24)
                if m == 3:
                    # proj gated by (m3, qq) finishes: qq0 pops at chunk
                    # 5, qq1 at ~14, qq2 at ~30
                    if i < 6:
                        return 0
                    if i < 14:
                        return min(8, i - 5)
                    if i < 30:
                        return min(16, 8 + (i - 13))
                    return min(n, 16 + (i - 29) * 2)
                return (i * n) // 40 + 1

            # qk for m=0 up front
            for c in range(NC512):
                q_chunk(0, c)
                k_chunk(0, c)

            DEPTH = 2
            pend = []
            ytq = {}     # (m, qq) -> [yt_even, yt_odd]

            def pop_pv():
                pm, pqq, pj, ppt = pend.pop(0)
                key = (pm, pqq)
                if pj == 0:
                    ytq[key] = [psY.tile([MV, W], dt32, tag="yt", name="yt")
                                for _ in range(2)]
                pv_chunk(pm, pqq, pj, ppt, ytq[key])
                if pj == pqq * 4 + 3:  # last kpos tile of this quarter
                    finish(2 * pm, pqq, ytq[key][0])
                    finish(2 * pm + 1, pqq, ytq[key][1])

            for m in range(NP):
                fillers = pair_fillers(m)
                fi = 0
                i = 0
                for qq in range(4):
                    for j in range(qq * 4 + 4):
                        while fi < len(fillers) and \
                                fi < wanted(m, i, len(fillers)):
                            fillers[fi]()
                            fi += 1
                        pt = s_chunk(m, qq, j)
                        pend.append((m, qq, j, pt))
                        if len(pend) > DEPTH:
                            pop_pv()
                        i += 1
                while fi < len(fillers):
                    fillers[fi]()
                    fi += 1
            while pend:
                pop_pv()
            # remaining proj: tt 12..15 (needs last quarter finishes)
            for tt in range(12, TT):
                for nn in range(2):
                    proj_chunk(tt, nn)

    nc.compile()
    return nc


def make_core_inputs(x, W_attn, b_attn, W_proj, n_cores=8, HC=8, D=64):
    """Host-side sharding: per-core input dicts."""
    B, T, C = x.shape
    CO = HC * D
    NP = CO // P
    in_maps = []
    for c in range(n_cores):
        b = c // (n_cores // B)
        h0 = (c % (n_cores // B)) * HC
        lo = h0 * D
        bq = b_attn[lo:lo + CO]
        bk = b_attn[C + lo:C + lo + CO]
        bv = b_attn[2 * C + lo:2 * C + lo + CO]
        bf = _bf16_np()
        in_maps.append({
            "xt": np.ascontiguousarray(x[b].T).astype(bf),
            "wq": np.ascontiguousarray(W_attn[:, lo:lo + CO]).astype(bf),
            "wk": np.ascontiguousarray(W_attn[:, C + lo:C + lo + CO]).astype(bf),
            "wv": np.ascontiguousarray(W_attn[:, 2 * C + lo:2 * C + lo + CO]).astype(bf),
            "bq": np.ascontiguousarray(bq.reshape(NP, P).T),
            "bk": np.ascontiguousarray(bk.reshape(NP, P).T),
            "bvb": np.tile(bv[None, :], (P, 1)),
            "ones": np.ones((P, (T // P) * HC), _bf16_np()),
            "wp": np.ascontiguousarray(W_proj[lo:lo + CO, :]),
        })
    return in_maps


_CACHE = {}


def _get_program():
    if "nc" not in _CACHE:
        _CACHE["nc"] = build_program()
    return _CACHE["nc"]


def run_on_cores(x, W_attn, b_attn, W_proj, b_proj, trace=False):
    """Returns (full output [B,T,C], BassKernelResults)."""
    from concourse.bass_utils import run_bass_kernel_spmd

    x = np.asarray(x, np.float32)
    W_attn = np.asarray(W_attn, np.float32)
    b_attn = np.asarray(b_attn, np.float32)
    W_proj = np.asarray(W_proj, np.float32)
    b_proj = np.asarray(b_proj, np.float32)

    nc = _get_program()
    in_maps = make_core_inputs(x, W_attn, b_attn, W_proj)
    res = run_bass_kernel_spmd(nc, in_maps, core_ids=list(range(8)), trace=trace)
    B, T, C = x.shape
    out = np.empty((B, T, C), np.float32)
    for b in range(B):
        out[b] = (res.results[2 * b]["out"].astype(np.float32)
                  + res.results[2 * b + 1]["out"].astype(np.float32)
                  + b_proj[None, :])
    return out, res


def kernel(x, W_attn, b_attn, W_proj, b_proj):
    out, _ = run_on_cores(x, W_attn, b_attn, W_proj, b_proj, trace=False)
    return out
